# revision 33
# baseline (speedup 1.0000x reference)
"""Trainium2 Bass kernel for nn_AMMaskedLinear.

Math: the reference's per-sample weight mask is separable:
    weight_mask[b,o,i] = pl[b,i] * ph[b,o] * S[o,i]
with
    present[b,v] = any_j(hidden_rank[b,j] == v)            (v in 0..32)
    pl[b,i] = present[b, r_low[i]]  & (r_low[i]  != 0)
    ph[b,o] = present[b, r_high[o]] & (r_high[o] != 0)
    om[b,o] = present[b, r_high[o]]
    S[o,i]  = (r_low[i] <= r_high[o])
so
    y[b,o]   = ph[b,o] * sum_i (S[o,i]*direction[o,i]) * (pl[b,i]*x[b,i])
    out[b,o] = cscale_b[o] * y[b,o] + om[b,o] * cbias_b[o]
(The Linear(1,out) layers applied to zeros contribute exactly their bias.)

Distribution: OUT (o) is sharded across the 8 NeuronCores; direction is the
only large tensor and each core only touches its own 128-row slice.  All
host-side work is layout/dtype transformation only (transpose / reshape /
broadcast / int->float cast / constant tables); every arithmetic op on the
problem's data runs on the device.

Device pipeline per core (SPMD, no collectives):
  1. presence bit-pack: w0 = 1 << min(hr,16), w1 = 1 << relu(hr-15) on a
     [128, 512] layout (partition = (half, b)), OR-tree along free axis,
     OR the two halves -> packed [64, 2] int32.
  2. extract 33 presence bits -> pres [64, 33] f32, PE-transpose -> [33, 64].
  3. one-hot matmuls gather presence:  plT[j,b], cbias*om[k,b], cscale*ph[k,b].
  4. E[j,k] = direction^T[j,k] * (r_high[k] >= r_low[j])   (fused stt)
  5. YT[k,b] = sum_j E[j,k] * (xT[j,b] * plT[j,b])         (PE, fp32)
  6. outT[k,b] = (cscale*ph)*YT + (cbias*om)               (DVE)

Engine/sync constraints honored throughout (neuronxcc wait-slot limits):
each instruction may carry at most ONE cross-engine sem wait, and the final
drain must cover few distinct sems — hence exactly two input DMAs, no
gpsimd usage, and probe copies that "absorb" DMA waits onto the DVE early.
"""

import numpy as np

B, IN, OUT, D = 64, 1024, 1024, 32
NCORES = 8
KSH = OUT // NCORES  # 128 outputs per core
NT = IN // 128       # 8 contraction tiles

# aux128 [128, AW] f32 column layout (DMA 1, needed first)
A_HR = 0             # [128, 512]  hidden_rank, int32 bitcast
A_RLPP = 512         # [128, 8]    r_low as f32, partition-major
A_RHBB = 520         # [128, 128]  r_high shard bcast over partitions
A_SHAMT = 648        # [64, 33]    bit-extraction shifts, int32 bitcast
A_IDENT = 681        # [64, 64]    identity for PE transpose
A_VIOTA = 745        # [33, 1]     permuted value index, f32
AW = 746

# big128 [128, BW] f32 column layout (DMA 2)
B_XT = 0             # [128, 8*64]   x^T, tile-major
B_DIRT = 512         # [128, 8*128]  direction^T shard, tile-major
B_V33 = 1536         # [33, 1408]    vals33 block (rows 33..127 are padding)
V_RL = 0             # [33, 1024] r_low bcast
V_RH = 1024          # [33, 128]  r_high shard bcast
V_CS = 1152          # [33, 128]  cscale shard bcast
V_CB = 1280          # [33, 128]  cbias shard bcast
VW = 1408
BW = B_V33 + VW      # 2944

_cached = {}


def _build_nc():
    import contextlib

    import concourse.bass as bass
    import concourse.mybir as mybir

    f32 = mybir.dt.float32
    i32 = mybir.dt.int32
    Alu = mybir.AluOpType

    nc = bass.Bass()

    aux_h = nc.declare_dram_parameter("aux128", [128, AW], f32, isOutput=False)
    big_h = nc.declare_dram_parameter("big128", [128, BW], f32, isOutput=False)
    out_h = nc.declare_dram_parameter("out", [KSH, B], f32, isOutput=True)

    ctx = contextlib.ExitStack()

    def sb(name, shape, dt=f32):
        return ctx.enter_context(nc.sbuf_tensor(name, shape, dt))[:]

    def ps(name, shape):
        return ctx.enter_context(nc.psum_tensor(name, shape, f32))[:]

    with ctx:
        aux_t = sb("aux_t", [128, AW])
        big_t = sb("big_t", [128, BW])
        ones_t = sb("ones_t", [128, 2, 512], i32)
        amt_t = sb("amt_t", [128, 2, 512], i32)
        w_t = sb("w_t", [128, 2, 512], i32)
        packed_hi_t = sb("packed_hi_t", [64, 2], i32)
        packed_t = sb("packed_t", [64, 2], i32)
        p33_t = sb("p33_t", [64, 33], i32)
        pres_t = sb("pres_t", [64, 33])
        ident_t = sb("ident_t", [64, 64])
        presT_t = sb("presT_t", [33, 64])
        ohlow_t = sb("ohlow_t", [33, IN])
        ohhigh_t = sb("ohhigh_t", [33, KSH])
        ohhs_t = sb("ohhs_t", [33, KSH])
        ohhb_t = sb("ohhb_t", [33, KSH])
        xlT_t = sb("xlT_t", [128, NT, B])
        E_t = sb("E_t", [128, NT, KSH])
        phs_t = sb("phs_t", [KSH, B])
        oms_t = sb("oms_t", [KSH, B])
        y1_t = sb("y1_t", [KSH, B])
        outT_t = sb("outT_t", [KSH, B])

        presT_ps = ps("presT_ps", [33, 64])
        plT_ps = ps("plT_ps", [128, NT, B])
        oms_ps = ps("oms_ps", [KSH, B])
        phs_ps = ps("phs_ps", [KSH, B])
        Y_ps = ps("Y_ps", [KSH, B])

        hr_ap = aux_t[:, A_HR : A_HR + 512].bitcast(i32)
        rlowpp_ap = aux_t[:, A_RLPP : A_RLPP + NT]
        rhighbb_ap = aux_t[:, A_RHBB : A_RHBB + KSH]
        shamt_ap = aux_t[0:64, A_SHAMT : A_SHAMT + 33].bitcast(i32)
        ident_ap = aux_t[0:64, A_IDENT : A_IDENT + 64]
        viota_ap = aux_t[0:33, A_VIOTA : A_VIOTA + 1]
        xT_ap = big_t[:, B_XT : B_XT + NT * B].rearrange("p (t b) -> p t b", t=NT)
        dirT_ap = big_t[:, B_DIRT : B_DIRT + NT * KSH].rearrange(
            "p (t k) -> p t k", t=NT
        )
        v33 = big_t[0:33, B_V33 : B_V33 + VW]
        rlow33_ap = v33[:, V_RL : V_RL + IN]
        rhigh33_ap = v33[:, V_RH : V_RH + KSH]
        cs33_ap = v33[:, V_CS : V_CS + KSH]
        cb33_ap = v33[:, V_CB : V_CB + KSH]

        aux_sem = ctx.enter_context(nc.semaphore("aux_sem"))
        big_sem = ctx.enter_context(nc.semaphore("big_sem"))
        out_sem = ctx.enter_context(nc.semaphore("out_sem"))
        dve_sem = ctx.enter_context(nc.semaphore("dve_sem"))
        pe_sem = ctx.enter_context(nc.semaphore("pe_sem"))
        block = ctx.enter_context(nc.Block())

        @block.sync
        def _(sync):
            sync.dma_start(out=aux_t, in_=aux_h[:, :]).then_inc(aux_sem, 16)
            sync.dma_start(out=big_t, in_=big_h[:, :]).then_inc(big_sem, 16)
            sync.wait_ge(dve_sem, 6)
            sync.dma_start(out=out_h[:, :], in_=outT_t).then_inc(out_sem, 16)
            sync.wait_ge(out_sem, 16)

        @block.vector
        def _(vector):
            # ---- presence bit-pack on [128=(half,b), 512] ----
            nc.vector.memset(ones_t, 1)
            vector.wait_ge(aux_sem, 16)  # aux128 landed
            # word0 bits 0..15 <- values 0..15 (bit16 = garbage, ignored)
            nc.vector.tensor_scalar(
                out=amt_t[:, 0, :], in0=hr_ap, scalar1=16, scalar2=None,
                op0=Alu.min,
            )
            # word1 bits 1..17 <- values 16..32 (bit0 = garbage, ignored)
            nc.vector.tensor_scalar(
                out=amt_t[:, 1, :], in0=hr_ap, scalar1=15, scalar2=0,
                op0=Alu.subtract, op1=Alu.max,
            )
            vector.drain()  # DVE same-engine RAW edges need explicit drains
            nc.vector.tensor_tensor(
                out=w_t, in0=ones_t, in1=amt_t, op=Alu.logical_shift_left
            )
            # OR-tree along free axis: 512 -> 1
            s = 256
            while s >= 1:
                vector.drain()
                nc.vector.tensor_tensor(
                    out=w_t[:, :, 0:s], in0=w_t[:, :, 0:s],
                    in1=w_t[:, :, s : 2 * s], op=Alu.bitwise_or,
                )
                s //= 2
            # combine the two row-halves (equal-base-partition rule: stage
            # the upper half through a copy first)
            vector.drain()
            nc.vector.tensor_copy(out=packed_hi_t, in_=w_t[64:128, :, 0])
            vector.drain()
            nc.vector.tensor_tensor(
                out=packed_t, in0=w_t[0:64, :, 0], in1=packed_hi_t,
                op=Alu.bitwise_or,
            )
            # ---- extract 33 presence bits (permuted column order: col c ->
            # value c+1 for c in 0..31, col 32 -> value 0; shamt matches) ----
            vector.drain()
            nc.vector.tensor_copy(
                out=p33_t[:, 0:15], in_=packed_t[:, 0:1].broadcast_to((64, 15))
            )
            nc.vector.tensor_copy(
                out=p33_t[:, 15:32], in_=packed_t[:, 1:2].broadcast_to((64, 17))
            )
            nc.vector.tensor_copy(out=p33_t[:, 32:33], in_=packed_t[:, 0:1])
            vector.drain()
            nc.vector.tensor_tensor(
                out=p33_t, in0=p33_t, in1=shamt_ap, op=Alu.logical_shift_right
            )
            vector.drain()
            nc.vector.tensor_scalar(
                out=p33_t, in0=p33_t, scalar1=1, scalar2=None,
                op0=Alu.bitwise_and,
            )
            vector.drain()
            nc.vector.tensor_copy(out=pres_t, in_=p33_t)
            nc.vector.tensor_copy(out=ident_t, in_=ident_ap).then_inc(dve_sem, 1)
            # dve=1: pres_t + ident_t ready (PE can transpose)

            # ---- one-hots over the (permuted) value axis ----
            vector.wait_ge(big_sem, 16)  # big128 landed
            nc.vector.tensor_scalar(
                out=ohlow_t, in0=rlow33_ap, scalar1=viota_ap,
                scalar2=None, op0=Alu.is_equal,
            )
            nc.vector.tensor_scalar(
                out=ohhigh_t, in0=rhigh33_ap, scalar1=viota_ap,
                scalar2=None, op0=Alu.is_equal,
            )
            vector.drain()
            nc.vector.tensor_mul(out=ohhs_t, in0=ohhigh_t, in1=cs33_ap)
            nc.vector.tensor_mul(
                out=ohhb_t, in0=ohhigh_t, in1=cb33_ap
            ).then_inc(dve_sem, 1)
            # dve=2: one-hots ready

            vector.wait_ge(pe_sem, 1)  # presT_ps ready
            nc.vector.tensor_copy(out=presT_t, in_=presT_ps).then_inc(dve_sem, 1)
            # dve=3: presT in SBUF (PE can run the gather matmuls)

            # ---- masked weights: E[j,(t),k] = (rhigh[k] >= rlow[j])*dirT ----
            for t in range(NT):
                ins = nc.vector.scalar_tensor_tensor(
                    out=E_t[:, t, :], in0=rhighbb_ap,
                    scalar=rlowpp_ap[:, t : t + 1],
                    in1=dirT_ap[:, t, :],
                    op0=Alu.is_ge, op1=Alu.mult,
                )
            ins.then_inc(dve_sem, 1)
            # dve=4: E ready

            vector.wait_ge(pe_sem, 2)  # plT_ps ready
            nc.vector.tensor_mul(
                out=xlT_t, in0=xT_ap, in1=plT_ps
            ).then_inc(dve_sem, 1)
            # dve=5: xlT ready (PE can run the main matmul)

            vector.wait_ge(pe_sem, 3)  # oms/phs ready
            nc.vector.tensor_copy(out=phs_t, in_=phs_ps)
            nc.vector.tensor_copy(out=oms_t, in_=oms_ps)
            vector.wait_ge(pe_sem, 4)  # Y ready
            vector.drain()
            nc.vector.tensor_mul(out=y1_t, in0=phs_t, in1=Y_ps)
            vector.drain()
            nc.vector.tensor_add(
                out=outT_t, in0=y1_t, in1=oms_t
            ).then_inc(dve_sem, 1)
            # dve=6: output ready in SBUF

        @block.tensor
        def _(tensor):
            tensor.wait_ge(dve_sem, 1)
            nc.tensor.transpose(presT_ps, pres_t, ident_t).then_inc(pe_sem, 1)
            # pe=1: presT_ps ready
            tensor.wait_ge(dve_sem, 3)  # one-hots (2) + presT sbuf (3)
            # plT[j, b] = present[b, r_low[j]] * (r_low[j] != 0)
            for t in range(NT):
                ins = nc.tensor.matmul(
                    plT_ps[:, t, :],
                    ohlow_t[0:32, t * 128 : (t + 1) * 128],
                    presT_t[0:32, :],
                )
            ins.then_inc(pe_sem, 1)
            # pe=2: plT ready
            # oms[k,b] = cbias[k]*om;  phs[k,b] = cscale[k]*ph
            nc.tensor.matmul(oms_ps, ohhb_t[0:33, :], presT_t[0:33, :])
            nc.tensor.matmul(
                phs_ps, ohhs_t[0:32, :], presT_t[0:32, :]
            ).then_inc(pe_sem, 1)
            # pe=3: oms/phs ready
            tensor.wait_ge(dve_sem, 5)  # E (4) + xlT (5)
            for t in range(NT):
                ins = nc.tensor.matmul(
                    Y_ps, E_t[:, t, :], xlT_t[:, t, :],
                    start=(t == 0), stop=(t == NT - 1),
                )
            ins.then_inc(pe_sem, 1)
            # pe=4: Y ready

    return nc


def _host_tables():
    """Input-independent constant tables (shift amounts, identity, iota)."""
    shamt = np.empty((64, 33), np.int32)
    shamt[:, 0:15] = np.arange(1, 16)[None, :]    # values 1..15 in word0
    shamt[:, 15:32] = np.arange(1, 18)[None, :]   # values 16..32 in word1
    shamt[:, 32] = 0                              # value 0 in word0
    ident = np.eye(64, dtype=np.float32)
    viota = np.empty((33, 1), np.float32)
    viota[0:32, 0] = np.arange(1, 33)
    viota[32, 0] = 0.0
    return shamt, ident, viota


def _prep_in_maps(inputs):
    """Host-side sharding: layout / dtype transforms only, no arithmetic."""
    x = np.ascontiguousarray(np.asarray(inputs["x"], dtype=np.float32))
    hr = np.ascontiguousarray(np.asarray(inputs["hidden_rank"], dtype=np.int32))
    r_low = np.asarray(inputs["r_low"], dtype=np.int32)
    r_high = np.asarray(inputs["r_high"], dtype=np.int32)
    direction = np.asarray(inputs["direction"], dtype=np.float32)
    cscale_b = np.asarray(inputs["cscale_b"], dtype=np.float32)
    cbias_b = np.asarray(inputs["cbias_b"], dtype=np.float32)

    # partition p = h*64 + b, free = s: hr2[h*64+b, s] = hr[b, h*512+s]
    hr2 = hr.reshape(B, 2, 512).transpose(1, 0, 2).reshape(128, 512)
    # xT3[p, t, b] = x[b, t*128+p]
    xT3 = x.T.reshape(NT, 128, B).transpose(1, 0, 2)
    rlowf = r_low.astype(np.float32)
    rhighf = r_high.astype(np.float32)
    shamt, ident, viota = _host_tables()

    aux = np.zeros((128, AW), np.float32)
    aux[:, A_HR : A_HR + 512] = hr2.view(np.float32)
    aux[:, A_RLPP : A_RLPP + NT] = rlowf.reshape(NT, 128).T
    aux[0:64, A_SHAMT : A_SHAMT + 33] = shamt.view(np.float32)
    aux[0:64, A_IDENT : A_IDENT + 64] = ident
    aux[0:33, A_VIOTA : A_VIOTA + 1] = viota

    in_maps = []
    for c in range(NCORES):
        sl = slice(c * KSH, (c + 1) * KSH)
        rh = rhighf[sl]
        auxc = aux.copy()
        auxc[:, A_RHBB : A_RHBB + KSH] = rh[None, :]
        big = np.zeros((128, BW), np.float32)
        big[:, B_XT : B_XT + NT * B] = xT3.reshape(128, -1)
        big[:, B_DIRT : B_DIRT + NT * KSH] = (
            direction[sl, :].T.reshape(NT, 128, KSH).transpose(1, 0, 2).reshape(128, -1)
        )
        big[0:33, B_V33 + V_RL : B_V33 + V_RL + IN] = rlowf[None, :]
        big[0:33, B_V33 + V_RH : B_V33 + V_RH + KSH] = rh[None, :]
        big[0:33, B_V33 + V_CS : B_V33 + V_CS + KSH] = cscale_b[sl][None, :]
        big[0:33, B_V33 + V_CB : B_V33 + V_CB + KSH] = cbias_b[sl][None, :]
        in_maps.append({"aux128": auxc, "big128": big})
    return in_maps


def _run(inputs, trace=False, **kw):
    from concourse.bass_utils import run_bass_kernel_spmd

    if "nc" not in _cached:
        _cached["nc"] = _build_nc()
    nc = _cached["nc"]
    in_maps = _prep_in_maps(inputs)
    res = run_bass_kernel_spmd(
        nc, in_maps, core_ids=list(range(NCORES)), trace=trace, **kw
    )
    out = np.concatenate([np.asarray(r["out"]).T for r in res.results], axis=1)
    return out.astype(np.float32), res


def kernel(**inputs):
    out, _ = _run(inputs, trace=False)
    return out


# revision 36
# speedup vs baseline: 1.4199x; 1.4199x over previous
"""Trainium2 Bass kernel for nn_AMMaskedLinear.

Math: the reference's per-sample weight mask is separable:
    weight_mask[b,o,i] = pl[b,i] * ph[b,o] * S[o,i]
with
    present[b,v] = any_j(hidden_rank[b,j] == v)            (v in 0..32)
    pl[b,i] = present[b, r_low[i]]  & (r_low[i]  != 0)
    ph[b,o] = present[b, r_high[o]] & (r_high[o] != 0)
    om[b,o] = present[b, r_high[o]]
    S[o,i]  = (r_low[i] <= r_high[o])
so
    y[b,o]   = ph[b,o] * sum_i (S[o,i]*direction[o,i]) * (pl[b,i]*x[b,i])
    out[b,o] = cscale_b[o] * y[b,o] + om[b,o] * cbias_b[o]
(The Linear(1,out) layers applied to zeros contribute exactly their bias.)

Distribution: OUT (o) is sharded across the 8 NeuronCores; direction is the
only large tensor and each core only touches its own 128-row slice.  All
host-side work is layout/dtype transformation only (transpose / reshape /
broadcast / int->float cast / constant tables); every arithmetic op on the
problem's data runs on the device.

Device pipeline per core (SPMD, raw bass, no collectives):
  1. presence bit-pack: w0 = 1 << min(hr,16), w1 = 1 << relu(hr-15) on a
     [128, 512] layout (partition = (half, b)), OR-tree along free axis,
     OR the two halves -> packed [64, 2] int32.
  2. extract 33 presence bits -> pres [64, 33] bf16, PE-transpose -> [33,64].
  3. one-hot matmuls (bf16, exact on 0/1) gather presence:
     plT[j,b], cbias*om[k,b], cscale*ph[k,b].
  4. E[j,k] = direction^T[j,k] * (r_high[k] >= r_low[j])   (fused stt, bf16)
  5. YT[k,b] = sum_j E[j,k] * (xT[j,b] * plT[j,b])         (PE bf16, f32 acc)
  6. outT[k,b] = (cscale*ph)*YT + (cbias*om)               (DVE f32)

Raw bass (not Tile): this neuronxcc build allows only ONE sync-wait per
instruction, which Tile's auto-generated multi-wait drains violate.  Sync
is explicit: one semaphore per DMA, dve_sem/pe_sem milestone chains, and
standalone wait_ge instructions.  DVE same-engine RAW edges carry explicit
drain() (the sim race detector requires them; HW needs the pipe flush).
"""

import numpy as np

B, IN, OUT, D = 64, 1024, 1024, 32
NCORES = 8
KSH = OUT // NCORES  # 128 outputs per core
NT = IN // 128       # 8 contraction tiles

# aux128 [128, AW] f32 column layout (DMA 1, needed first)
A_HR = 0             # [128, 512]  hidden_rank, int32 bitcast
A_RLPP = 512         # [128, 8]    r_low as f32, partition-major
A_RHBB = 520         # [128, 128]  r_high shard bcast over partitions
A_SHAMT = 648        # [64, 33]    bit-extraction shifts, int32 bitcast
A_IDENT = 681        # [64, 64]    bf16 identity (32 f32 cols), rows 0:64
A_VIOTA = 713        # [33, 1]     permuted value index, f32
A_ONES = 714         # [128, 1]    int32 ones, bitcast
AW = 715

# big128 [128, BW] f32 column layout (DMA 2)
B_XT = 0             # [128, 8*64]   x^T, tile-major
B_DIRT = 512         # [128, 8*128]  direction^T shard, tile-major
B_V33 = 1536         # [33, 1408] bf16 vals (704 f32 cols), rows 33.. padding
V_RL = 0             # [33, 1024] r_low bcast          (bf16 units)
V_RH = 1024          # [33, 128]  r_high shard bcast
V_CS = 1152          # [33, 128]  cscale shard bcast
V_CB = 1280          # [33, 128]  cbias shard bcast
VW = 1408            # bf16 units = 704 f32 cols
BW = B_V33 + VW // 2  # 2240

_cached = {}


def _build_nc():
    import contextlib

    import concourse.bass as bass
    import concourse.mybir as mybir

    f32 = mybir.dt.float32
    bf16 = mybir.dt.bfloat16
    i32 = mybir.dt.int32
    Alu = mybir.AluOpType

    nc = bass.Bass()

    aux_h = nc.declare_dram_parameter("aux128", [128, AW], f32, isOutput=False)
    big_h = nc.declare_dram_parameter("big128", [128, BW], f32, isOutput=False)
    out_h = nc.declare_dram_parameter("out", [KSH, B], f32, isOutput=True)

    ctx = contextlib.ExitStack()

    def sb(name, shape, dt=f32):
        return ctx.enter_context(nc.sbuf_tensor(name, shape, dt))[:]

    def ps(name, shape, dt=f32):
        return ctx.enter_context(nc.psum_tensor(name, shape, dt))[:]

    with ctx:
        aux_t = sb("aux_t", [128, AW])
        big_t = sb("big_t", [128, BW])
        amt_t = sb("amt_t", [128, 2, 512], i32)
        w_t = sb("w_t", [128, 2, 512], i32)
        packed_hi_t = sb("packed_hi_t", [64, 2], i32)
        packed_t = sb("packed_t", [64, 2], i32)
        p33_t = sb("p33_t", [64, 33], i32)
        pres_t = sb("pres_t", [64, 33], bf16)
        ident_t = sb("ident_t", [64, 64], bf16)
        presT_t = sb("presT_t", [33, 64], bf16)
        ohlow_t = sb("ohlow_t", [33, IN], bf16)
        ohhigh_t = sb("ohhigh_t", [33, KSH], bf16)
        ohhs_t = sb("ohhs_t", [33, KSH], bf16)
        ohhb_t = sb("ohhb_t", [33, KSH], bf16)
        xlT_t = sb("xlT_t", [128, NT, B], bf16)
        E_t = sb("E_t", [128, NT, KSH], bf16)
        phs_t = sb("phs_t", [KSH, B])
        oms_t = sb("oms_t", [KSH, B])
        y1_t = sb("y1_t", [KSH, B])
        outT_t = sb("outT_t", [KSH, B])

        presT_ps = ps("presT_ps", [33, 64], bf16)
        plT_ps = ps("plT_ps", [128, NT, B])
        oms_ps = ps("oms_ps", [KSH, B])
        phs_ps = ps("phs_ps", [KSH, B])
        Y_ps = ps("Y_ps", [KSH, B])

        hr_ap = aux_t[:, A_HR : A_HR + 512].bitcast(i32)
        rlowpp_ap = aux_t[:, A_RLPP : A_RLPP + NT]
        rhighbb_ap = aux_t[:, A_RHBB : A_RHBB + KSH]
        shamt_ap = aux_t[0:64, A_SHAMT : A_SHAMT + 33].bitcast(i32)
        ident_ap = aux_t[0:64, A_IDENT : A_IDENT + 32].bitcast(bf16)
        viota_ap = aux_t[0:33, A_VIOTA : A_VIOTA + 1]
        ones_ap = aux_t[:, A_ONES : A_ONES + 1].bitcast(i32)
        xT_ap = big_t[:, B_XT : B_XT + NT * B].rearrange("p (t b) -> p t b", t=NT)
        dirT_ap = big_t[:, B_DIRT : B_DIRT + NT * KSH].rearrange(
            "p (t k) -> p t k", t=NT
        )
        v33 = big_t[0:33, B_V33 : B_V33 + VW // 2].bitcast(bf16)
        rlow33_ap = v33[:, V_RL : V_RL + IN]
        rhigh33_ap = v33[:, V_RH : V_RH + KSH]
        cs33_ap = v33[:, V_CS : V_CS + KSH]
        cb33_ap = v33[:, V_CB : V_CB + KSH]

        aux_sem = ctx.enter_context(nc.semaphore("aux_sem"))
        big_sem = ctx.enter_context(nc.semaphore("big_sem"))
        out_sem = ctx.enter_context(nc.semaphore("out_sem"))
        dve_sem = ctx.enter_context(nc.semaphore("dve_sem"))
        pe_sem = ctx.enter_context(nc.semaphore("pe_sem"))
        block = ctx.enter_context(nc.Block())

        @block.sync
        def _(sync):
            sync.dma_start(out=aux_t, in_=aux_h[:, :]).then_inc(aux_sem, 16)
            sync.dma_start(out=big_t, in_=big_h[:, :]).then_inc(big_sem, 16)
            sync.wait_ge(dve_sem, 6)
            sync.dma_start(out=out_h[:, :], in_=outT_t).then_inc(out_sem, 16)
            sync.wait_ge(out_sem, 16)

        @block.vector
        def _(vector):
            # ---- presence bit-pack on [128=(half,b), 512] ----
            vector.wait_ge(aux_sem, 16)  # aux128 landed
            # word0 bits 0..15 <- values 0..15 (bit16 = garbage, ignored)
            nc.vector.tensor_scalar(
                out=amt_t[:, 0, :], in0=hr_ap, scalar1=16, scalar2=None,
                op0=Alu.min,
            )
            # word1 bits 1..17 <- values 16..32 (bit0 = garbage, ignored)
            nc.vector.tensor_scalar(
                out=amt_t[:, 1, :], in0=hr_ap, scalar1=15, scalar2=0,
                op0=Alu.subtract, op1=Alu.max,
            )
            vector.drain()  # DVE same-engine RAW edges need explicit drains
            nc.vector.tensor_tensor(
                out=w_t, in0=ones_ap[:, :, None].broadcast_to((128, 2, 512)),
                in1=amt_t, op=Alu.logical_shift_left,
            )
            # OR-tree along free axis: 512 -> 1
            s = 256
            while s >= 1:
                vector.drain()
                nc.vector.tensor_tensor(
                    out=w_t[:, :, 0:s], in0=w_t[:, :, 0:s],
                    in1=w_t[:, :, s : 2 * s], op=Alu.bitwise_or,
                )
                s //= 2
            # combine the two row-halves (equal-base-partition rule: stage
            # the upper half through a copy first)
            vector.drain()
            nc.vector.tensor_copy(out=packed_hi_t, in_=w_t[64:128, :, 0])
            vector.drain()
            nc.vector.tensor_tensor(
                out=packed_t, in0=w_t[0:64, :, 0], in1=packed_hi_t,
                op=Alu.bitwise_or,
            )
            # ---- extract 33 presence bits (permuted column order: col c ->
            # value c+1 for c in 0..31, col 32 -> value 0; shamt matches) ----
            vector.drain()
            nc.vector.tensor_tensor(
                out=p33_t[:, 0:15],
                in0=packed_t[:, 0:1].broadcast_to((64, 15)),
                in1=shamt_ap[:, 0:15], op=Alu.logical_shift_right,
            )
            nc.vector.tensor_tensor(
                out=p33_t[:, 15:32],
                in0=packed_t[:, 1:2].broadcast_to((64, 17)),
                in1=shamt_ap[:, 15:32], op=Alu.logical_shift_right,
            )
            nc.vector.tensor_copy(out=p33_t[:, 32:33], in_=packed_t[:, 0:1])
            vector.drain()
            nc.vector.tensor_scalar(
                out=p33_t, in0=p33_t, scalar1=1, scalar2=None,
                op0=Alu.bitwise_and,
            )
            vector.drain()
            nc.vector.tensor_copy(out=pres_t, in_=p33_t)
            nc.vector.tensor_copy(out=ident_t, in_=ident_ap).then_inc(dve_sem, 1)
            # dve=1: pres_t + ident_t ready (PE can transpose)

            # ---- one-hots over the (permuted) value axis: row r -> value
            # r+1 for r in 0..31, row 32 -> value 0 (viota matches) ----
            vector.wait_ge(big_sem, 16)  # big128 landed
            nc.vector.tensor_scalar(
                out=ohlow_t, in0=rlow33_ap, scalar1=viota_ap,
                scalar2=None, op0=Alu.is_equal,
            )
            vector.wait_ge(pe_sem, 1)  # presT_ps ready
            nc.vector.tensor_copy(out=presT_t, in_=presT_ps).then_inc(dve_sem, 1)
            # dve=2: ohlow + presT ready (PE can gather plT)

            nc.vector.tensor_scalar(
                out=ohhigh_t, in0=rhigh33_ap, scalar1=viota_ap,
                scalar2=None, op0=Alu.is_equal,
            )
            vector.drain()
            # fold cscale/cbias into the one-hots so the gather matmuls emit
            # cscale*ph and cbias*om directly
            nc.vector.tensor_mul(out=ohhs_t, in0=ohhigh_t, in1=cs33_ap)
            nc.vector.tensor_mul(
                out=ohhb_t, in0=ohhigh_t, in1=cb33_ap
            ).then_inc(dve_sem, 1)
            # dve=3: scaled one-hots ready (PE can compute oms/phs)

            # ---- masked weights: E[j,(t),k] = (rhigh[k] >= rlow[j])*dirT ----
            for t in range(NT):
                ins = nc.vector.scalar_tensor_tensor(
                    out=E_t[:, t, :], in0=rhighbb_ap,
                    scalar=rlowpp_ap[:, t : t + 1],
                    in1=dirT_ap[:, t, :],
                    op0=Alu.is_ge, op1=Alu.mult,
                )
            ins.then_inc(dve_sem, 1)
            # dve=4: E ready

            vector.wait_ge(pe_sem, 2)  # plT_ps ready
            nc.vector.tensor_mul(
                out=xlT_t, in0=xT_ap, in1=plT_ps
            ).then_inc(dve_sem, 1)
            # dve=5: xlT ready (PE can run the main matmul)

            vector.wait_ge(pe_sem, 3)  # oms/phs ready
            nc.vector.tensor_copy(out=phs_t, in_=phs_ps)
            nc.vector.tensor_copy(out=oms_t, in_=oms_ps)
            vector.wait_ge(pe_sem, 4)  # Y ready
            vector.drain()
            nc.vector.tensor_mul(out=y1_t, in0=phs_t, in1=Y_ps)
            vector.drain()
            nc.vector.tensor_add(
                out=outT_t, in0=y1_t, in1=oms_t
            ).then_inc(dve_sem, 1)
            # dve=6: output ready in SBUF

        @block.tensor
        def _(tensor):
            tensor.wait_ge(dve_sem, 1)
            nc.tensor.transpose(presT_ps, pres_t, ident_t).then_inc(pe_sem, 1)
            # pe=1: presT_ps ready
            tensor.wait_ge(dve_sem, 2)
            # plT[j, b] = present[b, r_low[j]] * (r_low[j] != 0)
            for t in range(NT):
                ins = nc.tensor.matmul(
                    plT_ps[:, t, :],
                    ohlow_t[0:32, t * 128 : (t + 1) * 128],
                    presT_t[0:32, :],
                )
            ins.then_inc(pe_sem, 1)
            # pe=2: plT ready
            tensor.wait_ge(dve_sem, 3)
            # oms[k,b] = cbias[k]*om;  phs[k,b] = cscale[k]*ph
            nc.tensor.matmul(oms_ps, ohhb_t[0:33, :], presT_t[0:33, :])
            nc.tensor.matmul(
                phs_ps, ohhs_t[0:32, :], presT_t[0:32, :]
            ).then_inc(pe_sem, 1)
            # pe=3: oms/phs ready
            tensor.wait_ge(dve_sem, 5)  # E (4) + xlT (5)
            for t in range(NT):
                ins = nc.tensor.matmul(
                    Y_ps, E_t[:, t, :], xlT_t[:, t, :],
                    start=(t == 0), stop=(t == NT - 1),
                )
            ins.then_inc(pe_sem, 1)
            # pe=4: Y ready

    return nc


def _host_tables():
    """Input-independent constant tables (shift amounts, identity, iota)."""
    import ml_dtypes

    shamt = np.empty((64, 33), np.int32)
    shamt[:, 0:15] = np.arange(1, 16)[None, :]    # values 1..15 in word0
    shamt[:, 15:32] = np.arange(1, 18)[None, :]   # values 16..32 in word1
    shamt[:, 32] = 0                              # value 0 in word0
    ident = np.eye(64, dtype=ml_dtypes.bfloat16)
    viota = np.empty((33, 1), np.float32)
    viota[0:32, 0] = np.arange(1, 33)
    viota[32, 0] = 0.0
    return shamt, ident, viota


def _prep_in_maps(inputs):
    """Host-side sharding: layout / dtype transforms only, no arithmetic."""
    import ml_dtypes

    bf = ml_dtypes.bfloat16
    x = np.ascontiguousarray(np.asarray(inputs["x"], dtype=np.float32))
    hr = np.ascontiguousarray(np.asarray(inputs["hidden_rank"], dtype=np.int32))
    r_low = np.asarray(inputs["r_low"], dtype=np.int32)
    r_high = np.asarray(inputs["r_high"], dtype=np.int32)
    direction = np.asarray(inputs["direction"], dtype=np.float32)
    cscale_b = np.asarray(inputs["cscale_b"], dtype=np.float32)
    cbias_b = np.asarray(inputs["cbias_b"], dtype=np.float32)

    # partition p = h*64 + b, free = s: hr2[h*64+b, s] = hr[b, h*512+s]
    hr2 = hr.reshape(B, 2, 512).transpose(1, 0, 2).reshape(128, 512)
    # xT3[p, t, b] = x[b, t*128+p]
    xT3 = x.T.reshape(NT, 128, B).transpose(1, 0, 2)
    rlowf = r_low.astype(np.float32)
    rhighf = r_high.astype(np.float32)
    shamt, ident, viota = _host_tables()

    aux = np.zeros((128, AW), np.float32)
    aux[:, A_HR : A_HR + 512] = hr2.view(np.float32)
    aux[:, A_RLPP : A_RLPP + NT] = rlowf.reshape(NT, 128).T
    aux[0:64, A_SHAMT : A_SHAMT + 33] = shamt.view(np.float32)
    aux[0:64, A_IDENT : A_IDENT + 32] = ident.view(np.float32)
    aux[0:33, A_VIOTA : A_VIOTA + 1] = viota
    aux[:, A_ONES] = np.float32(np.int32(1).view(np.float32))

    in_maps = []
    for c in range(NCORES):
        sl = slice(c * KSH, (c + 1) * KSH)
        rh = rhighf[sl]
        auxc = aux.copy()
        auxc[:, A_RHBB : A_RHBB + KSH] = rh[None, :]
        big = np.zeros((128, BW), np.float32)
        big[:, B_XT : B_XT + NT * B] = xT3.reshape(128, -1)
        big[:, B_DIRT : B_DIRT + NT * KSH] = (
            direction[sl, :].T.reshape(NT, 128, KSH).transpose(1, 0, 2).reshape(128, -1)
        )
        vals = np.zeros((33, VW), bf)
        vals[:, V_RL : V_RL + IN] = rlowf[None, :].astype(bf)
        vals[:, V_RH : V_RH + KSH] = rh[None, :].astype(bf)
        vals[:, V_CS : V_CS + KSH] = cscale_b[sl][None, :].astype(bf)
        vals[:, V_CB : V_CB + KSH] = cbias_b[sl][None, :].astype(bf)
        big[0:33, B_V33 : B_V33 + VW // 2] = vals.view(np.float32)
        in_maps.append({"aux128": auxc, "big128": big})
    return in_maps


def _run(inputs, trace=False, **kw):
    from concourse.bass_utils import run_bass_kernel_spmd

    if "nc" not in _cached:
        _cached["nc"] = _build_nc()
    nc = _cached["nc"]
    in_maps = _prep_in_maps(inputs)
    res = run_bass_kernel_spmd(
        nc, in_maps, core_ids=list(range(NCORES)), trace=trace, **kw
    )
    out = np.concatenate([np.asarray(r["out"]).T for r in res.results], axis=1)
    return out.astype(np.float32), res


def kernel(**inputs):
    out, _ = _run(inputs, trace=False)
    return out
